# revision 48
# baseline (speedup 1.0000x reference)
"""Trainium2 Bass kernel for CompactKroneckerFusion.

Math: out = relu(LN((x1@S1 * x2@S2) @ W + b)), where S1/S2 are count-sketch
matrices (exactly one +-1 per row). The product (x1@S1)*(x2@S2) is nonzero
only on sketch buckets hit by BOTH sketches (~117 of 8192 for these shapes),
so the whole computation collapses to small gathers + tiny dense GEMMs:

  J      = {buckets hit by both sketches}            (|J| = nj)
  x1g    = x1 columns that land in J, transposed     [n1, B]
  A1     = (col -> bucket-in-J) +-1 scatter matrix   [n1, nj]
  sk1^T  = A1^T @ x1g                                [nj, B]   (PE matmul)
  ck^T   = sk1^T * sk2^T  (+ ones row for bias)      [nj+1, B] (DVE)
  h      = ck^T^T @ [W[J]; b]                        [B, OUT]  (PE matmul)
  out    = relu((h - mu) * rsqrt(var + eps) * gamma + beta)    (DVE + ACT)

Sharding: data-parallel over batch across 8 cores; A1/A2/W[J] replicated.
Host side only extracts indices / gathers columns (cheap, O(input size));
all FLOPs above run on-device.
"""

import os
import sys
from contextlib import ExitStack

import numpy as np

_REPO = "/opt/trn_rl_repo"
if _REPO not in sys.path:
    sys.path.insert(0, _REPO)

import concourse.bass as bass  # noqa: E402
import concourse.mybir as mybir  # noqa: E402
import concourse.tile as tile  # noqa: E402

N_CORES = 8
PMAX = 128  # partitions / max matmul K and M
NMAX = 512  # max matmul moving free dim (one PSUM bank of f32)
F32 = mybir.dt.float32
LN_EPS = 1e-5

# PE fp32 matmul runs at 4 cycles/row; float32r (same 32-bit storage,
# reduced-precision PE path) runs at 1 cycle/row for moving dim >= 256.
# All matmul operands (x panels, A scatter matrices, Wg, ck) are declared
# float32r end-to-end; PSUM accumulation stays fp32.
MM_DT = os.environ.get("BASS_KERNEL_MM_DT", "float32r")
XDT = mybir.dt.float32r if MM_DT == "float32r" else mybir.dt.float32

LAST_EXEC_TIME_NS = None
LAST_TRACE_PATH = None
LAST_RESULTS = None


# Trim the TileContext exit epilogue: the stock version emits
# drain + barrier + semaphore-clear + barrier (~2 us).  The semaphore clears
# only matter for re-executing a NEFF whose semaphores must start from
# zero; every kernel() call compiles and loads a fresh NEFF, so one
# drain + barrier suffices.
def _install_lean_exit():
    if getattr(tile.TileContext, "_lean_exit", False):
        return
    from concourse.tile import ScopedClock

    def _drain_and_barrier(self, tick_clock, wait_clock):
        nc = self.nc
        drain_inst = nc.sync.drain()
        wait_clock.add_sem_waits(
            drain_inst.ins, ScopedClock({None: tick_clock.global_clock})
        )
        popped = nc._tile_sem_poison_stack.pop()
        assert popped is self._sem_poison
        sem_nums = [s.num for s in self.sems.allocated().values()]
        nc._state.prepend_free_semaphores(sem_nums)
        for poison_set in nc._tile_sem_poison_stack:
            poison_set.update(sem_nums)

    tile.TileContext._drain_and_barrier = _drain_and_barrier
    tile.TileContext._lean_exit = True


_install_lean_exit()


# Skip the all-engine barrier Bass.__init__ emits after its const-AP
# memsets: nothing in this kernel reads those constants before Tile's own
# dependency-tracked syncs (the one float-bias const feeds only the
# throwaway ACT-table warmup op).
def _bass_no_init_barrier():
    if getattr(bass.Bass, "_no_init_barrier", False):
        return
    orig_init = bass.Bass.__init__

    def patched_init(self, *a, **k):
        orig = bass.Bass.all_engine_barrier
        bass.Bass.all_engine_barrier = lambda self_, **kw: None
        try:
            orig_init(self, *a, **k)
        finally:
            bass.Bass.all_engine_barrier = orig

    bass.Bass.__init__ = patched_init
    bass.Bass._no_init_barrier = True


_bass_no_init_barrier()


# ---------------------------------------------------------------------------
# Toolchain workaround: this walrus build rejects instructions carrying more
# than one sync wait ("Too many sync wait commands").  After Tile lowering,
# hoist surplus waits onto same-engine NoOps inserted immediately before the
# owning instruction — the engine stalls on the carriers first, so ordering
# semantics are preserved.
# ---------------------------------------------------------------------------
def _split_multi_waits(nc, max_waits=1):
    n_split = 0
    for f in nc.m.functions:
        for blk in f.blocks:
            insts = blk.instructions
            out = []
            for inst in insts:
                si = inst.sync_info
                waits = list(si.on_wait) if si is not None and si.on_wait else []
                if len(waits) > max_waits:
                    extra = waits[: len(waits) - max_waits]
                    si.on_wait[:] = waits[len(waits) - max_waits :]
                    for k, w in enumerate(extra):
                        nop = mybir.InstNoOp(
                            name=f"{inst.name}-wc{k}", ins=[], outs=[]
                        )
                        nop.engine = inst.engine
                        nop.sync_info = mybir.SyncInfo(on_wait=[w], on_update=[])
                        out.append(nop)
                        n_split += 1
                out.append(inst)
            insts[:] = out
    return n_split


# ---------------------------------------------------------------------------
# Host-side restructuring
# ---------------------------------------------------------------------------
def _extract_sketch(S):
    """Count-sketch matrix -> (bucket index, sign) per input dim."""
    S = np.asarray(S, dtype=np.float32)
    idx = np.abs(S).argmax(1).astype(np.int64)
    s = S[np.arange(S.shape[0]), idx]
    return idx, s


def _plan_side(idx, s, pos, jchunks):
    """Group contributing input columns by J-chunk and split into K-subchunks.

    Returns (src_cols, dest_rows, per-chunk subchunk descriptors, n_rows).
    Each subchunk: (row_offset_in_xg, A_matrix [len_sub, njc]).  Chunk row
    bases are 32-aligned so matmul partition reads stay legal.
    """
    keep = (s != 0) & (pos[idx] >= 0)
    cols = np.where(keep)[0]
    p = pos[idx[cols]]
    chunk_of = p // PMAX
    order = np.lexsort((cols, chunk_of))
    cols = cols[order]
    p = p[order]
    sg = s[cols]

    per_chunk = []
    dest_rows = np.empty(len(cols), np.int64)
    row_base = 0
    for ci, (c0, njc) in enumerate(jchunks):
        ccols = np.where(chunk_of[order] == ci)[0]  # positions within `cols`
        dest_rows[ccols] = row_base + np.arange(len(ccols))
        subs = []
        for s0 in range(0, len(ccols), PMAX):
            sel = ccols[s0 : s0 + PMAX]
            A = np.zeros((len(sel), njc), np.float32)
            A[np.arange(len(sel)), p[sel] - c0] = sg[sel]
            subs.append((row_base + s0, A))
        per_chunk.append(subs)
        row_base += len(ccols)
        row_base = (row_base + 31) // 32 * 32
    return cols, dest_rows, per_chunk, max(row_base, 1)


def _prepare(x1, x2, S1, S2, W, b, ln_gamma, ln_beta):
    x1 = np.ascontiguousarray(np.asarray(x1, np.float32))
    x2 = np.ascontiguousarray(np.asarray(x2, np.float32))
    W = np.asarray(W, np.float32)
    b = np.asarray(b, np.float32)
    ln_gamma = np.asarray(ln_gamma, np.float32)
    ln_beta = np.asarray(ln_beta, np.float32)

    B = x1.shape[0]
    OUT = W.shape[1]
    SK = S1.shape[1]
    assert OUT <= NMAX, "OUT dim > 512 not supported by this kernel"
    assert B % (N_CORES * PMAX) == 0

    idx1, s1 = _extract_sketch(S1)
    idx2, s2 = _extract_sketch(S2)
    J = np.intersect1d(idx1[s1 != 0], idx2[s2 != 0])
    nj = len(J)
    pos = np.full(SK, -1, np.int64)
    pos[J] = np.arange(nj)

    jchunks = [(c0, min(PMAX, nj - c0)) for c0 in range(0, nj, PMAX)]

    cols1, dest1, sub1, nr1 = _plan_side(idx1, s1, pos, jchunks)
    cols2, dest2, sub2, nr2 = _plan_side(idx2, s2, pos, jchunks)

    x1g = np.zeros((nr1, B), np.float32)
    if len(cols1):
        x1g[dest1] = x1[:, cols1].T
    x2g = np.zeros((nr2, B), np.float32)
    if len(cols2):
        x2g[dest2] = x2[:, cols2].T

    # W rows for each chunk; bias folded in as a ones-row contraction on the
    # last chunk (or as its own chunk when there's no room / no buckets).
    # Compute-engine SBUF writes must start at a 32-aligned partition, so a
    # bias-carrying chunk is padded to 128 rows: ck rows [96:128) are preset
    # to 1.0 (the product overwrites rows up to njc), and Wg rows beyond njc
    # are zero except the bias row at njc — the spurious ones hit zero rows.
    chunks = []
    for ci, (c0, njc) in enumerate(jchunks):
        chunks.append(
            {"njc": njc, "has_bias": False, "nrows": njc,
             "Wg": W[J[c0 : c0 + njc], :], "sub1": sub1[ci], "sub2": sub2[ci]}
        )
    if not chunks or chunks[-1]["njc"] == PMAX:
        chunks.append(
            {"njc": 0, "has_bias": True, "nrows": 0,
             "Wg": np.zeros((0, OUT), np.float32), "sub1": [], "sub2": []}
        )
    ch = chunks[-1]
    ch["has_bias"] = True
    pad = np.zeros((PMAX - ch["njc"], OUT), np.float32)
    pad[0] = b
    ch["Wg"] = np.concatenate([ch["Wg"], pad], 0)
    ch["nrows"] = PMAX
    for ch in chunks:
        ch["Wg"] = np.ascontiguousarray(ch["Wg"], np.float32)

    affine_trivial = bool(np.all(ln_gamma == 1.0) and np.all(ln_beta == 0.0))

    # Prepend the A scatter matrices as extra columns of the x panels: the
    # first x-piece DMA then delivers A and the first batch columns together,
    # so a single completion semaphore gates the first sketch matmul.
    a1blk = np.zeros((nr1, sum(A.shape[1] for subs in sub1 for (_, A) in subs)),
                     np.float32)
    a2blk = np.zeros((nr2, sum(A.shape[1] for subs in sub2 for (_, A) in subs)),
                     np.float32)
    aoff1, aoff2 = {}, {}
    off = 0
    for ci, subs in enumerate(sub1):
        for si, (r0, A) in enumerate(subs):
            a1blk[r0 : r0 + A.shape[0], off : off + A.shape[1]] = A
            aoff1[(ci, si)] = off
            off += A.shape[1]
    off = 0
    for ci, subs in enumerate(sub2):
        for si, (r0, A) in enumerate(subs):
            a2blk[r0 : r0 + A.shape[0], off : off + A.shape[1]] = A
            aoff2[(ci, si)] = off
            off += A.shape[1]

    return {
        "B": B,
        "OUT": OUT,
        "B_core": B // N_CORES,
        "n1": x1g.shape[0],
        "n2": x2g.shape[0],
        "x1g": x1g,
        "x2g": x2g,
        "a1blk": a1blk,
        "a2blk": a2blk,
        "aoff1": aoff1,
        "aoff2": aoff2,
        "chunks": chunks,
        "affine_trivial": affine_trivial,
        "gvec": np.ascontiguousarray(ln_gamma[None, :]),
        "bvec": np.ascontiguousarray(ln_beta[None, :]),
    }


# ---------------------------------------------------------------------------
# Device program
# ---------------------------------------------------------------------------
def _build_program(plan):
    B_core = plan["B_core"]
    OUT = plan["OUT"]
    chunks = plan["chunks"]
    BT = NMAX if B_core % NMAX == 0 else PMAX
    assert B_core % BT == 0 and BT % PMAX == 0
    n_t = B_core // BT
    n_m = BT // PMAX

    nc = bass.Bass()

    a1w = plan["a1blk"].shape[1]
    a2w = plan["a2blk"].shape[1]
    x1g_d = nc.dram_tensor(
        "x1aug", [plan["n1"], a1w + B_core], XDT, kind="ExternalInput"
    )
    x2g_d = nc.dram_tensor(
        "x2aug", [plan["n2"], a2w + B_core], XDT, kind="ExternalInput"
    )
    wg_d = [
        nc.dram_tensor(f"Wg_{ci}", list(ch["Wg"].shape), XDT, kind="ExternalInput")
        for ci, ch in enumerate(chunks)
    ]
    if not plan["affine_trivial"]:
        g_d = nc.dram_tensor("gvec", [1, OUT], F32, kind="ExternalInput")
        be_d = nc.dram_tensor("bvec", [1, OUT], F32, kind="ExternalInput")
    y_d = nc.dram_tensor("y", [B_core, OUT], F32, kind="ExternalOutput")

    with tile.TileContext(nc) as tc, ExitStack() as ctx:
        consts = ctx.enter_context(tc.tile_pool(name="consts", bufs=1))
        xin = ctx.enter_context(tc.tile_pool(name="xin", bufs=6))
        ckp = ctx.enter_context(tc.tile_pool(name="ck", bufs=7))
        pss = ctx.enter_context(tc.tile_pool(name="pss", bufs=1, space="PSUM"))
        psh = ctx.enter_context(tc.tile_pool(name="psh", bufs=6, space="PSUM"))
        stat = ctx.enter_context(tc.tile_pool(name="stat", bufs=8))
        outp = ctx.enter_context(tc.tile_pool(name="outp", bufs=8))

        wg_sb = []
        for ci, ch in enumerate(chunks):
            t = consts.tile(list(ch["Wg"].shape), XDT, tag=f"Wg_{ci}")
            nc.gpsimd.dma_start(out=t[:], in_=wg_d[ci][:])
            wg_sb.append(t)
        eps_t = consts.tile([PMAX, 1], F32, tag="eps")
        nc.vector.memset(eps_t[:], LN_EPS)
        warm_t = consts.tile([PMAX, 1], F32, tag="warm")
        nc.scalar.activation(
            warm_t[:], eps_t[:], mybir.ActivationFunctionType.Relu
        )
        if not plan["affine_trivial"]:
            g_sb = consts.tile([PMAX, OUT], F32, tag="gamma")
            nc.gpsimd.dma_start(out=g_sb[:], in_=g_d[:].to_broadcast([PMAX, OUT]))
            be_sb = consts.tile([PMAX, OUT], F32, tag="beta")
            nc.gpsimd.dma_start(out=be_sb[:], in_=be_d[:].to_broadcast([PMAX, OUT]))

        # x panels stream in pieces: DMA completion latency is roughly
        # 1.4 us + 2x the transfer slice, so the first pieces are small to
        # let the PE start early, then widen.  The very first piece of each
        # panel also carries that panel's A columns.  x1 rides the sync
        # HWDGE ring, x2 the scalar ring (all its pieces land before ACT
        # compute starts).  Output stores ride the (otherwise idle) sync
        # ring.
        def pieces_for(ti):
            if ti == 0 and BT >= 512:
                return [(0, 128), (128, 128), (256, 256)]
            return [(p0, min(256, BT - p0)) for p0 in range(0, BT, 256)]

        a1_sb = {}
        a2_sb = {}
        any_buckets = any(ch["njc"] > 0 for ch in chunks)
        for ti in range(n_t):
            # ck is built per piece (separate tiles) so each h-matmul only
            # waits on the piece covering its row range, not the whole
            # BT-wide panel.
            ck_pieces = {}
            for pi, (poff, pw) in enumerate(pieces_for(ti)):
                first = ti == 0 and pi == 0
                aw1 = a1w if first else 0
                aw2 = a2w if first else 0
                x1t = xin.tile(
                    [plan["n1"], aw1 + pw], XDT, tag="x1f" if first else "x1"
                )
                nc.sync.dma_start(
                    out=x1t[:], in_=x1g_d[:, bass.ds(a1w + ti * BT + poff - aw1, aw1 + pw)]
                )
                x2t = xin.tile(
                    [plan["n2"], aw2 + pw], XDT, tag="x2f" if first else "x2"
                )
                nc.scalar.dma_start(
                    out=x2t[:], in_=x2g_d[:, bass.ds(a2w + ti * BT + poff - aw2, aw2 + pw)]
                )
                if first:
                    for ci, ch in enumerate(chunks):
                        for si, (r0, A) in enumerate(ch["sub1"]):
                            o = plan["aoff1"][(ci, si)]
                            a1_sb[(ci, si)] = x1t[
                                r0 : r0 + A.shape[0], o : o + A.shape[1]
                            ]
                        for si, (r0, A) in enumerate(ch["sub2"]):
                            o = plan["aoff2"][(ci, si)]
                            a2_sb[(ci, si)] = x2t[
                                r0 : r0 + A.shape[0], o : o + A.shape[1]
                            ]
                for ci, ch in enumerate(chunks):
                    njc = ch["njc"]
                    ck = ckp.tile([ch["nrows"], pw], XDT, tag=f"ck{ci}")
                    ck_pieces[(ci, pi)] = (poff, pw, ck)
                    if ch["has_bias"]:
                        ones0 = (njc // 32) * 32
                        nc.gpsimd.memset(ck[ones0:PMAX, :].bitcast(F32), 1.0)
                    if njc == 0:
                        continue
                    ps1 = pss.tile([njc, pw], F32, tag=f"ps1_{ci}")
                    for si, (r0, A) in enumerate(ch["sub1"]):
                        nc.tensor.matmul(
                            ps1[:],
                            a1_sb[(ci, si)],
                            x1t[r0 : r0 + A.shape[0], aw1 : aw1 + pw],
                            start=(si == 0),
                            stop=(si == len(ch["sub1"]) - 1),
                        )
                    ps2 = pss.tile([njc, pw], F32, tag=f"ps2_{ci}")
                    for si, (r0, A) in enumerate(ch["sub2"]):
                        nc.tensor.matmul(
                            ps2[:],
                            a2_sb[(ci, si)],
                            x2t[r0 : r0 + A.shape[0], aw2 : aw2 + pw],
                            start=(si == 0),
                            stop=(si == len(ch["sub2"]) - 1),
                        )
                    sk1 = ckp.tile([njc, pw], F32, tag=f"sk1_{ci}")
                    nc.scalar.copy(sk1[:], ps1[:])
                    nc.vector.tensor_mul(ck[0:njc, :], sk1[:], ps2[:])

            # h = ck^T @ Wg per row-tile, with a per-tile LN+relu chain.
            # The normalize+relu pass alternates between the ACT engine and
            # a DVE+GpSimd pair so no single engine serializes the epilogue.
            pieces = pieces_for(ti)

            # h = ck^T @ Wg per row-tile.  The sqrt/reciprocal/negate scalar
            # chain is batched over PAIRS of row-tiles: half the small-op
            # count on the ACT/DVE queues, without serializing all four
            # tiles' relus behind the last tile's stats.
            mi = 0
            while mi < n_m:
                pr = min(2, n_m - mi)
                mvp = stat.tile([PMAX, pr, 2], F32, tag="mvp")
                phs = []
                for j in range(pr):
                    m0 = (mi + j) * PMAX
                    pidx = next(
                        i for i, (po, pw) in enumerate(pieces)
                        if po <= m0 < po + pw
                    )
                    ph = psh.tile([PMAX, OUT], F32, tag="ph")
                    for ci, ch in enumerate(chunks):
                        poff, pw, ck = ck_pieces[(ci, pidx)]
                        nc.tensor.matmul(
                            ph[:],
                            ck[:, m0 - poff : m0 - poff + PMAX],
                            wg_sb[ci][:],
                            start=(ci == 0),
                            stop=(ci == len(chunks) - 1),
                        )
                    stats = stat.tile([PMAX, 6], F32, tag="stats")
                    nc.vector.bn_stats(stats[:], ph[:])
                    nc.vector.bn_aggr(mvp[:, j, :], stats[:])
                    phs.append(ph)
                stdp = stat.tile([PMAX, pr], F32, tag="stdp")
                nc.scalar.activation(
                    stdp[:],
                    mvp[:, :, 1],
                    mybir.ActivationFunctionType.Sqrt,
                    bias=eps_t[:],
                )
                rstdp = stat.tile([PMAX, pr], F32, tag="rstdp")
                nc.vector.reciprocal(rstdp[:], stdp[:])
                nmrp = stat.tile([PMAX, pr], F32, tag="nmrp")
                nc.vector.tensor_tensor(
                    out=nmrp[:],
                    in0=mvp[:, :, 0],
                    in1=rstdp[:],
                    op=mybir.AluOpType.mult,
                )
                nc.vector.tensor_scalar_mul(nmrp[:], nmrp[:], -1.0)
                for j in range(pr):
                    mj = mi + j
                    out_t = outp.tile([PMAX, OUT], F32, tag="out")
                    if plan["affine_trivial"]:
                        nc.scalar.activation(
                            out_t[:],
                            phs[j][:],
                            mybir.ActivationFunctionType.Relu,
                            bias=nmrp[:, j : j + 1],
                            scale=rstdp[:, j : j + 1],
                        )
                    else:
                        tmp = outp.tile([PMAX, OUT], F32, tag="tmp")
                        nc.vector.tensor_scalar(
                            out=tmp[:],
                            in0=phs[j][:],
                            scalar1=mvp[:, j, 0:1],
                            scalar2=rstdp[:, j : j + 1],
                            op0=mybir.AluOpType.subtract,
                            op1=mybir.AluOpType.mult,
                        )
                        nc.vector.tensor_mul(tmp[:], tmp[:], g_sb[:])
                        nc.vector.tensor_add(tmp[:], tmp[:], be_sb[:])
                        nc.scalar.activation(
                            out_t[:], tmp[:], mybir.ActivationFunctionType.Relu
                        )
                    last = ti == n_t - 1 and mj == n_m - 1
                    if last and OUT % 2 == 0:
                        half = OUT // 2
                        nc.sync.dma_start(
                            out=y_d[ti * BT + mj * PMAX :][:PMAX, :half],
                            in_=out_t[:, :half],
                        )
                        nc.sync.dma_start(
                            out=y_d[ti * BT + mj * PMAX :][:PMAX, half:],
                            in_=out_t[:, half:],
                        )
                    else:
                        nc.sync.dma_start(
                            out=y_d[ti * BT + mj * PMAX :][:PMAX, :],
                            in_=out_t[:],
                        )
                mi += pr

    return nc


# ---------------------------------------------------------------------------
# Entry point
# ---------------------------------------------------------------------------
def kernel(x1, x2, S1, S2, W, b, ln_gamma, ln_beta):
    global LAST_EXEC_TIME_NS, LAST_TRACE_PATH
    plan = _prepare(x1, x2, S1, S2, W, b, ln_gamma, ln_beta)
    nc = _build_program(plan)
    _split_multi_waits(nc)

    common = {}
    for ci, ch in enumerate(plan["chunks"]):
        common[f"Wg_{ci}"] = ch["Wg"]
    if not plan["affine_trivial"]:
        common["gvec"] = plan["gvec"]
        common["bvec"] = plan["bvec"]

    B_core = plan["B_core"]
    in_maps = []
    for c in range(N_CORES):
        m = dict(common)
        sl = slice(c * B_core, (c + 1) * B_core)
        m["x1aug"] = np.ascontiguousarray(
            np.concatenate([plan["a1blk"], plan["x1g"][:, sl]], axis=1)
        )
        m["x2aug"] = np.ascontiguousarray(
            np.concatenate([plan["a2blk"], plan["x2g"][:, sl]], axis=1)
        )
        in_maps.append(m)

    trace = os.environ.get("BASS_KERNEL_TRACE", "") == "1"
    kwargs = {}
    if trace:
        from concourse import bass_utils

        bass_utils.upload_artifacts = lambda tmpdir: "local://" + tmpdir
        kwargs["trace"] = True
        if os.environ.get("BASS_KERNEL_TRACE_ALL", "") == "1":
            kwargs["trace_cores"] = list(range(N_CORES))

    from concourse.bass_utils import run_bass_kernel_spmd

    res = run_bass_kernel_spmd(nc, in_maps, list(range(N_CORES)), **kwargs)
    if trace:
        global LAST_RESULTS
        LAST_RESULTS = res
        LAST_EXEC_TIME_NS = res.exec_time_ns
        LAST_TRACE_PATH = (
            res.instructions_and_trace[1] if res.instructions_and_trace else None
        )

    return np.concatenate([res.results[c]["y"] for c in range(N_CORES)], 0)
